# revision 51
# baseline (speedup 1.0000x reference)
"""Trainium2 Bass kernel for nn_MultiHeadAttention (B=2, T=2048, D=1024, H=16).

Sharding: 8 cores; core c owns head pair (2c, 2c+1) = output-channel slice
[c*128, (c+1)*128) of Wq/Wk/Wv columns and Wo rows (tensor parallel), both
batches. Host pre-transposes/packs x and weight slices into per-partition
contiguous layouts (large DMA descriptors); each core computes a partial
output projection over its 128 ctx channels; host sums the 8 partials
(replaces the all-reduce) and adds bo.

Per-core dataflow (f16 matmuls, moving N=512):
  QT/KT[e,t] projections (xT moving), VT projection + PE-transpose to V
  natural [t,e] with a fused ones-column for the softmax denominator;
  per (batch, 1024-wide q-pair): scoresT[k,q] = KT.T @ QT row-tiled 2 heads
  into 2-bank PSUM, exp on ACT over [128,1024] (scale=1/8 fused), ctx
  accumulation ctxU_aug[65,1024] = [V|1].T @ escT over 16 k-tiles,
  software-pipelined one k-tile behind exp so the PE never waits on the
  exp it just triggered; 1/s via DVE reciprocal_approx_fast + PE
  outer-product broadcast; out-proj partial [t,e] = ctxT.T @ WoT_slice
  streamed to DRAM.

Schedule: flash-style start (attention on chunk (0,0) begins right after
tchunk0's K/V/Q; later K/V groups emit inside the kt loop just before the
k-tiles that need them); b0-Q (tch 1-3) + all b1 projections drain one
item per kt into b0's attention; each chunk's finalize pipelines into the
next chunk's kt loop (stage1@kt4, out-proj@kt8/12); the final chunk reads
its PSUM accumulators directly and alternates tail copies across ACT/DVE.
Host pre-packs x/weights per-partition-contiguous (8KB/4KB DMA
descriptors) in f16; bias adds run on ACT (Identity + bias AP).
"""

import numpy as np

P = 128
D = 1024
BT = 4096
T = 2048
NB = 2
DC = 8    # D chunks of 128
TCH = 8   # 512-wide t-chunks over BT
KT = 16   # 128-wide k-tiles per batch
QC = 4    # 512-wide q-chunks per batch
NCORES = 8
DK = 64

_CACHE = {}


def _build(reps=1):
    import concourse.bass as bass
    import concourse.tile as tile
    from concourse import bacc, mybir
    from concourse.masks import make_identity

    f32 = mybir.dt.float32
    f32r = mybir.dt.float32r
    f16 = mybir.dt.float16
    Exp = mybir.ActivationFunctionType.Exp
    Copy = mybir.ActivationFunctionType.Copy
    Identity = mybir.ActivationFunctionType.Identity
    ds = bass.ds

    nc = bacc.Bacc("TRN2", target_bir_lowering=False, debug=False)

    # host-packed layouts: per-partition contiguous (big DMA descriptors)
    xt = nc.dram_tensor("xt", [P, TCH, DC, 512], f16, kind="ExternalInput").ap()
    wq = nc.dram_tensor("wq", [P, DC, P], f16, kind="ExternalInput").ap()
    wk = nc.dram_tensor("wk", [P, DC, P], f16, kind="ExternalInput").ap()
    wv = nc.dram_tensor("wv", [P, DC, P], f16, kind="ExternalInput").ap()
    wo = nc.dram_tensor("wo", [P, D], f16, kind="ExternalInput").ap()
    bqd = nc.dram_tensor("bq", [P, 1], f32, kind="ExternalInput").ap()
    bkd = nc.dram_tensor("bk", [P, 1], f32, kind="ExternalInput").ap()
    bvd = nc.dram_tensor("bv", [P, 1], f32, kind="ExternalInput").ap()
    out = nc.dram_tensor("out", [BT, D], f32, kind="ExternalOutput").ap()

    with tile.TileContext(nc) as tc:
        with (
            tc.tile_pool(name="const", bufs=1) as constp,
            tc.tile_pool(name="xtp", bufs=3) as xtp,
            tc.tile_pool(name="qkv", bufs=1) as qkvp,
            tc.tile_pool(name="vts", bufs=2) as vtsp,
            tc.tile_pool(name="esc", bufs=4) as escp,
            tc.tile_pool(name="ctx", bufs=8) as ctxp,
            tc.tile_pool(name="small", bufs=2) as smallp,
            tc.tile_pool(name="bsb", bufs=2) as bsbp,
            tc.tile_pool(name="psS", bufs=3, space="PSUM") as psS,
            tc.tile_pool(name="psC", bufs=2, space="PSUM") as psC,
        ):
            # ---- constants ----
            wq_sb = constp.tile([P, DC, P], f16, tag="wq")
            wk_sb = constp.tile([P, DC, P], f16, tag="wk")
            wv_sb = constp.tile([P, DC, P], f16, tag="wv")
            # bias DMAs are issued late (inside the rep loop) so the big
            # wk/xt transfers hit the DMA queues first
            bq_sb = constp.tile([P, 1], f32, tag="bq")
            bk_sb = constp.tile([P, 1], f32, tag="bk")
            bv_sb = constp.tile([P, 1], f32, tag="bv")
            ident_f = constp.tile([P, P], f32, tag="identf")
            make_identity(nc, ident_f)
            ident = constp.tile([P, P], f16, tag="ident")
            nc.vector.tensor_copy(ident, ident_f)
            ones_f32 = constp.tile([P, 512], f32, tag="ones_f32")
            nc.vector.memset(ones_f32, 1.0)
            ones_t = constp.tile([P, 512], f32r, tag="ones")
            nc.vector.tensor_copy(ones_t, ones_f32)
            wo_sb = constp.tile([P, D], f16, tag="wo")

            # ---- per-batch persistent tiles ----
            qt_sb = [
                qkvp.tile([P, T], f16, tag=f"qt{b}", name=f"qt{b}")
                for b in range(NB)
            ]
            kt_sb = [
                qkvp.tile([P, T], f16, tag=f"kt{b}", name=f"kt{b}")
                for b in range(NB)
            ]
            # V natural per batch per head, 65-wide blocks: [V(64) | ones]
            va_sb = [
                qkvp.tile([P, KT * 65], f16, tag=f"va{b}", name=f"va{b}")
                for b in range(NB)
            ]
            vb_sb = [
                qkvp.tile([P, KT * 65], f16, tag=f"vb{b}", name=f"vb{b}")
                for b in range(NB)
            ]
            ones_col = ones_f32[:, 0:KT].rearrange("p (k one) -> p k one", one=1)
            for b in range(NB):
                nc.vector.tensor_copy(
                    va_sb[b].rearrange("p (k c) -> p k c", c=65)[:, :, 64:65],
                    ones_col,
                )
                nc.vector.tensor_copy(
                    vb_sb[b].rearrange("p (k c) -> p k c", c=65)[:, :, 64:65],
                    ones_col,
                )

            for _rep in range(reps):

                def load_xtile(tch, split=False):
                    xtile = xtp.tile([P, DC, 512], f16, tag="xt", name="xtile")
                    if split:
                        # two DMAs so the first half-contraction of the first
                        # projection can start ~2us earlier
                        nc.sync.dma_start(xtile[:, 0:4], xt[:, tch, 0:4])
                        nc.sync.dma_start(xtile[:, 4:DC], xt[:, tch, 4:DC])
                    else:
                        nc.sync.dma_start(xtile, xt[:, tch])
                    return xtile

                def proj_w(tch, xtile, w_sb, b_sb, dst, half):
                    # half 0/1: 4 contraction chunks each; half 1 closes the
                    # accumulation group and writes dst
                    if half == 0:
                        ps = psS.tile([P, 512], f32, tag="sc", name="psw")
                        _proj_ps[(tch, id(w_sb))] = ps
                        for c in range(4):
                            nc.tensor.matmul(
                                ps, w_sb[:, c], xtile[:, c],
                                start=(c == 0), stop=False,
                            )
                    else:
                        ps = _proj_ps.pop((tch, id(w_sb)))
                        for c in range(4, DC):
                            nc.tensor.matmul(
                                ps, w_sb[:, c], xtile[:, c],
                                start=False, stop=(c == DC - 1),
                            )
                        # bias-add on ACT (Identity w/ per-partition bias):
                        # keeps the DVE free for the fin chains
                        nc.scalar.activation(dst, ps, Identity, bias=b_sb)

                def proj_v_tail(tch, vts, half):
                    # transpose VT -> V natural; 2 t-tiles per half
                    b = tch // 4
                    for tt in (0, 1) if half == 0 else (2, 3):
                        ktile = (tch % 4) * 4 + tt
                        pvt = psS.tile([P, P], f16, tag="sc", name="pvt")
                        nc.tensor.transpose(pvt, vts[:, ds(tt * P, P)], ident)
                        nc.vector.tensor_copy(
                            va_sb[b][:, ds(ktile * 65, DK)], pvt[:, 0:DK]
                        )
                        nc.vector.tensor_copy(
                            vb_sb[b][:, ds(ktile * 65, DK)], pvt[:, DK:P]
                        )

                _proj_ps = {}

                def proj_k(tch, xtile):
                    b = tch // 4
                    tloc = (tch % 4) * 512
                    dst = kt_sb[b][:, ds(tloc, 512)]
                    proj_w(tch, xtile, wk_sb, bk_sb, dst, 0)
                    proj_w(tch, xtile, wk_sb, bk_sb, dst, 1)

                def proj_v(tch, xtile):
                    vts = vtsp.tile([P, 512], f16, tag="vts", name="vts")
                    proj_w(tch, xtile, wv_sb, bv_sb, vts, 0)
                    proj_w(tch, xtile, wv_sb, bv_sb, vts, 1)
                    proj_v_tail(tch, vts, 0)
                    proj_v_tail(tch, vts, 1)

                def proj_q_thunks(tch, xtile):
                    b = tch // 4
                    tloc = (tch % 4) * 512
                    dst = qt_sb[b][:, ds(tloc, 512)]
                    return [
                        lambda: proj_w(tch, xtile, wq_sb, bq_sb, dst, 0),
                        lambda: proj_w(tch, xtile, wq_sb, bq_sb, dst, 1),
                    ]

                def proj_kv_thunks(tch, xtile):
                    b = tch // 4
                    tloc = (tch % 4) * 512
                    kdst = kt_sb[b][:, ds(tloc, 512)]
                    vts = vtsp.tile([P, 512], f16, tag="vts", name="vts")
                    return [
                        lambda: proj_w(tch, xtile, wk_sb, bk_sb, kdst, 0),
                        lambda: proj_w(tch, xtile, wk_sb, bk_sb, kdst, 1),
                        lambda: proj_w(tch, xtile, wv_sb, bv_sb, vts, 0),
                        lambda: proj_w(tch, xtile, wv_sb, bv_sb, vts, 1),
                        lambda: proj_v_tail(tch, vts, 0),
                        lambda: proj_v_tail(tch, vts, 1),
                    ]

                # pipelined finalize: stage1 (recip+bcast+normalize) and
                # stage2 (out-projection) of the previous chunk are emitted
                # inside the current chunk's kt loop to keep PE/ACT streams
                # dense.
                def fin_stage1(st, last=False):
                    # st carries ua/ub (SBUF copies) for pipelined chunks —
                    # copying frees the PSUM banks for the next chunk's
                    # accumulation. The final chunk passes the raw PSUM
                    # accumulators + the reciprocals already computed inside
                    # its attnV epilogue (no successor to stall).
                    if last:
                        b, qch, cxa, cxb, rf = st
                    else:
                        b, qch, cxa, cxb = st
                        rf = smallp.tile(
                            [P, 1024], f32, tag="recipf", name="rf"
                        )
                        # approx_fast breaks on partition-offset APs; run it
                        # over the full [0:65] range (rows 0:64 are unused
                        # junk, only the s-row 64 is consumed downstream)
                        nc.vector.reciprocal_approx_fast(
                            rf[0:65, 0:512], cxa[0:65, :]
                        )
                        nc.vector.reciprocal_approx_fast(
                            rf[0:65, 512:1024], cxb[0:65, :]
                        )
                    bc_sb = bsbp.tile([DK, 1024], f32, tag="bcs", name="bc_sb")
                    rr = smallp.tile([P, 1024], f32r, tag="recip", name="rr")
                    nc.vector.tensor_copy(rr[64:65, :], rf[64:65, :])
                    bc = psS.tile([P, 1024], f32, tag="sc", name="bc")
                    nc.tensor.matmul(
                        bc[0:DK, 0:512], ones_t[64:65, 0:DK],
                        rr[64:65, 0:512],
                        start=True, stop=True, tile_position=(64, 0),
                    )
                    nc.tensor.matmul(
                        bc[0:DK, 512:1024], ones_t[64:65, 0:DK],
                        rr[64:65, 512:1024],
                        start=True, stop=True, tile_position=(64, 0),
                    )
                    nc.vector.tensor_copy(bc_sb, bc[0:DK, :])
                    ctq = ctxp.tile([P, 512], f16, tag="ctq", name="ctq")
                    nc.vector.tensor_mul(
                        ctq[0:DK, :], cxa[0:DK, :], bc_sb[:, 0:512]
                    )
                    tmpb = bsbp.tile([DK, 512], f16, tag="tmpb", name="tmpb")
                    nc.vector.tensor_mul(
                        tmpb, cxb[0:DK, :], bc_sb[:, 512:1024]
                    )
                    nc.sync.dma_start(ctq[DK:P, :], tmpb)
                    return ctq

                def fin_stage2(st, ctq, tts, last=False):
                    b, qch = st[0], st[1]
                    del st
                    q0 = qch * 512
                    for tt in tts:
                        po = psS.tile([P, 1024], f32, tag="sc", name="po")
                        nc.tensor.matmul(
                            po[:, 0:512],
                            ctq[:, ds(tt * P, P)], wo_sb[:, 0:512],
                            start=True, stop=True,
                        )
                        nc.tensor.matmul(
                            po[:, 512:1024],
                            ctq[:, ds(tt * P, P)], wo_sb[:, 512:1024],
                            start=True, stop=True,
                        )
                        po_sb = escp.tile(
                            [P, 1024], f32, tag="posb", name="po_sb", bufs=3
                        )
                        if last and tt % 2 == 0:
                            # after the final exp the ACT engine is idle:
                            # alternate the tail's PSUM->SBUF copies across
                            # ACT and DVE to halve the serial drain
                            nc.scalar.activation(po_sb, po, Copy)
                        else:
                            nc.vector.tensor_copy(po_sb, po)
                        r0 = b * T + q0 + tt * P
                        nc.sync.dma_start(out[r0 : r0 + P, :], po_sb)

                pending = {"st": None}

                work_q = []
                fin2_q = []

                def drain_pending(kt):
                    # stage1 of the previous chunk runs at kt4; its four
                    # out-proj tiles go on fin2_q, drained one per slot only
                    # when no projection work remains. This migrates out-proj
                    # PE work out of the b0 window (where the PE is already
                    # oversubscribed) into b1's ACT-paced chunks (where the
                    # PE otherwise stalls every kt).
                    if pending["st"] is not None and kt == 4:
                        st = pending["st"]
                        ctq = fin_stage1(st)
                        for tt in range(4):
                            fin2_q.append((st, ctq, tt))
                        pending["st"] = None
                        return
                    if kt >= 1:
                        if work_q:
                            work_q.pop(0)()
                        elif fin2_q:
                            st, ctq, tt = fin2_q.pop(0)
                            fin_stage2(st, ctq, (tt,))

                def attn_chunk(b, qch, interleave=None, last=False):
                    # software-pipelined by one kt: attnV(kt-1) is emitted
                    # after scores(kt)/exp(kt) so the PE never sits waiting
                    # on the exp it just triggered. `interleave` maps kt ->
                    # thunk emitted inside the loop (flash-style prologue).
                    q0 = qch * 512
                    cxa = psC.tile([65, 512], f32, tag="cx", name="cxa")
                    cxb = psC.tile([65, 512], f32, tag="cx", name="cxb")
                    escs = {}

                    def emit_av(kt, rf=None):
                        esc = escs.pop(kt)
                        nc.tensor.matmul(
                            cxa,
                            va_sb[b][:, ds(kt * 65, 65)],
                            esc[:, 0:512],
                            start=(kt == 0), stop=(kt == KT - 1),
                        )
                        if rf is not None:
                            # final chunk: head A's reciprocal runs on the
                            # DVE while head B's attnV still streams on PE
                            nc.vector.reciprocal_approx_fast(
                                rf[0:65, 0:512], cxa[0:65, :]
                            )
                        nc.tensor.matmul(
                            cxb,
                            vb_sb[b][:, ds(kt * 65, 65)],
                            esc[:, 512:1024],
                            start=(kt == 0), stop=(kt == KT - 1),
                        )
                        if rf is not None:
                            nc.vector.reciprocal_approx_fast(
                                rf[0:65, 512:1024], cxb[0:65, :]
                            )

                    for kt in range(KT):
                        sc = psS.tile([P, 1024], f32, tag="sc", name="sc")
                        nc.tensor.matmul(
                            sc[:, 0:512],
                            kt_sb[b][0:DK, ds(kt * P, P)],
                            qt_sb[b][0:DK, ds(q0, 512)],
                            start=True, stop=True,
                        )
                        nc.tensor.matmul(
                            sc[:, 512:1024],
                            kt_sb[b][DK:P, ds(kt * P, P)],
                            qt_sb[b][DK:P, ds(q0, 512)],
                            start=True, stop=True,
                            tile_position=(64, 0),
                        )
                        esc = escp.tile([P, 1024], f16, tag="esc", name="esc")
                        nc.scalar.activation(esc, sc, Exp, scale=0.125)
                        escs[kt] = esc
                        if kt > 0:
                            emit_av(kt - 1)
                        if interleave is not None and kt in interleave:
                            interleave[kt]()
                        else:
                            drain_pending(kt)
                    if last:
                        rf = smallp.tile(
                            [P, 1024], f32, tag="recipf", name="rf"
                        )
                        emit_av(KT - 1, rf=rf)
                        return (b, qch, cxa, cxb, rf)
                    emit_av(KT - 1)
                    ua = bsbp.tile([65, 512], f32, tag="ua", name="ua")
                    nc.vector.tensor_copy(ua, cxa)
                    ub = bsbp.tile([65, 512], f32, tag="ub", name="ub")
                    nc.vector.tensor_copy(ub, cxb)
                    return (b, qch, ua, ub)

                # flash-style start: K/V/Q of tchunk0 only, then begin
                # attention on chunk (0,0) immediately; K/V of tchunks 1-3
                # are emitted inside the kt loop right before the k-tiles
                # that need them (kt 4g needs tchunk g).
                xtiles_b0 = {}
                if _rep == 0:
                    nc.sync.dma_start(wk_sb, wk)
                xtiles_b0[0] = load_xtile(0, split=True)
                if _rep == 0:
                    nc.sync.dma_start(wv_sb, wv)
                    nc.sync.dma_start(wq_sb, wq)
                    nc.sync.dma_start(bk_sb, bkd)
                    nc.sync.dma_start(bv_sb, bvd)
                    nc.sync.dma_start(bq_sb, bqd)
                proj_k(0, xtiles_b0[0])
                xtiles_b0[1] = load_xtile(1)
                proj_v(0, xtiles_b0[0])
                for th in proj_q_thunks(0, xtiles_b0[0]):
                    th()
                if _rep == 0:
                    nc.sync.dma_start(wo_sb, wo)

                # K/V groups split in half so the ACT engine's ~2-exp
                # lookahead covers each PE burst: K(g) lands the kt before
                # its first score needs it, V(g) one kt later (attnV runs
                # one kt behind scores)
                def flash_k(g):
                    def th():
                        if g + 1 < 4:
                            xtiles_b0[g + 1] = load_xtile(g + 1)
                        proj_k(g, xtiles_b0[g])
                    return th

                def flash_v(g):
                    def th():
                        proj_v(g, xtiles_b0[g])
                    return th

                flash = {
                    3: flash_k(1), 4: flash_v(1),
                    7: flash_k(2), 8: flash_v(2),
                    11: flash_k(3), 12: flash_v(3),
                }

                # defer b0 Q (tch 1-3) + all of b1's projections into b0's
                # attention. Q thunks resolve xtiles_b0[tch] lazily (tch 2/3
                # xtiles only load inside the flash interleave).
                def q_b0(tch, half):
                    def th():
                        proj_q_thunks(tch, xtiles_b0[tch])[half]()
                    return th

                xt_b1 = {}

                def mk_load(tch):
                    def th():
                        xt_b1[tch] = load_xtile(tch)
                    return th

                def mk_body(tch, n=8):
                    out = []
                    for i in range(min(n, 6)):
                        def th(tch=tch, i=i):
                            xti = xt_b1[tch]
                            if "thunks" not in xt_b1.setdefault(
                                f"t{tch}", {}
                            ):
                                xt_b1[f"t{tch}"]["thunks"] = (
                                    proj_kv_thunks(tch, xti)
                                    + proj_q_thunks(tch, xti)
                                )
                            xt_b1[f"t{tch}"]["thunks"][i]()
                        out.append(th)
                    for i in range(max(0, n - 6)):
                        def th2(tch=tch, i=i):
                            xt_b1[f"t{tch}"]["thunks"][6 + i]()
                        out.append(th2)
                    return out

                def mk_q(tch, half):
                    def th():
                        xt_b1[f"t{tch}"]["thunks"][6 + half]()
                    return th

                work_q.append(q_b0(1, 0))
                work_q.append(q_b0(1, 1))
                work_q.append(q_b0(2, 0))
                work_q.append(q_b0(2, 1))
                work_q.append(mk_load(4))
                work_q.append(mk_load(5))
                work_q.append(q_b0(3, 0))
                work_q.append(q_b0(3, 1))
                work_q.extend(mk_body(4))
                work_q.append(mk_load(6))
                work_q.extend(mk_body(5))
                work_q.append(mk_load(7))
                work_q.extend(mk_body(6))
                work_q.extend(mk_body(7))
                pending["st"] = attn_chunk(0, 0, interleave=flash)
                for qch in range(1, 4):
                    pending["st"] = attn_chunk(0, qch)
                while work_q:
                    work_q.pop(0)()
                for qch in range(3):
                    pending["st"] = attn_chunk(1, qch)
                pending["st"] = attn_chunk(1, 3, last=True)
                while fin2_q:
                    st2, ctq2, tt2 = fin2_q.pop(0)
                    fin_stage2(st2, ctq2, (tt2,))
                ctq = fin_stage1(pending["st"], last=True)
                fin_stage2(pending["st"], ctq, (0, 1, 2, 3), last=True)

    nc.compile()
    return nc


def _get_nc(reps=1):
    key = f"nc{reps}"
    if key not in _CACHE:
        _CACHE[key] = _build(reps)
    return _CACHE[key]


def kernel(x, Wq, bq, Wk, bk, Wv, bv, Wo, bo):
    from concourse.bass_utils import run_bass_kernel_spmd

    x = np.asarray(x, dtype=np.float32)
    Wq = np.asarray(Wq, dtype=np.float32)
    Wk = np.asarray(Wk, dtype=np.float32)
    Wv = np.asarray(Wv, dtype=np.float32)
    Wo = np.asarray(Wo, dtype=np.float32)
    bq = np.asarray(bq, dtype=np.float32)
    bk = np.asarray(bk, dtype=np.float32)
    bv = np.asarray(bv, dtype=np.float32)
    bo = np.asarray(bo, dtype=np.float32)

    B, Tl, Dl = x.shape
    x_flat = x.reshape(B * Tl, Dl)
    # [p, tch, c, tt] with value x_flat[tch*512 + tt, c*128 + p]
    xt = np.ascontiguousarray(
        x_flat.reshape(TCH, 512, DC, P).transpose(3, 0, 2, 1)
    ).astype(np.float16)

    def pack_w(Wslice_T):
        # [p, c, e] with value Wslice_T[c*128 + p, e]
        return np.ascontiguousarray(
            Wslice_T.reshape(DC, P, P).transpose(1, 0, 2)
        ).astype(np.float16)

    in_maps = []
    for c in range(NCORES):
        sl = slice(c * P, (c + 1) * P)
        in_maps.append(
            {
                "xt": xt,
                "wq": pack_w(np.ascontiguousarray(Wq[sl, :].T)),
                "wk": pack_w(np.ascontiguousarray(Wk[sl, :].T)),
                "wv": pack_w(np.ascontiguousarray(Wv[sl, :].T)),
                "wo": np.ascontiguousarray(Wo[:, sl].T).astype(np.float16),
                "bq": np.ascontiguousarray(bq[sl].reshape(P, 1)),
                "bk": np.ascontiguousarray(bk[sl].reshape(P, 1)),
                "bv": np.ascontiguousarray(bv[sl].reshape(P, 1)),
            }
        )

    nc = _get_nc()
    _CACHE["in_maps"] = in_maps
    res = run_bass_kernel_spmd(nc, in_maps, core_ids=list(range(NCORES)))
    acc = res.results[0]["out"].astype(np.float32)
    for c in range(1, NCORES):
        acc = acc + res.results[c]["out"]
    acc = acc + bo[None, :]
    return acc.reshape(B, Tl, Dl).astype(np.float32)


# revision 52
# speedup vs baseline: 1.1158x; 1.1158x over previous
"""Trainium2 Bass kernel for nn_MultiHeadAttention (B=2, T=2048, D=1024, H=16).

Sharding: 8 cores; core c owns head pair (2c, 2c+1) = output-channel slice
[c*128, (c+1)*128) of Wq/Wk/Wv columns and Wo rows (tensor parallel), both
batches. Host pre-transposes/packs x and weight slices into per-partition
contiguous layouts (large DMA descriptors); each core computes a partial
output projection over its 128 ctx channels; host sums the 8 partials
(replaces the all-reduce) and adds bo.

Per-core dataflow (f16 matmuls, moving N=512):
  QT/KT[e,t] projections (xT moving), VT projection + PE-transpose to V
  natural [t,e] with a fused ones-column for the softmax denominator;
  per (batch, 1024-wide q-pair): scoresT[k,q] = KT.T @ QT row-tiled 2 heads
  into 2-bank PSUM, exp on ACT over [128,1024] (scale=1/8 fused), ctx
  accumulation ctxU_aug[65,1024] = [V|1].T @ escT over 16 k-tiles,
  software-pipelined one k-tile behind exp so the PE never waits on the
  exp it just triggered; 1/s via DVE reciprocal_approx_fast + PE
  outer-product broadcast; out-proj partial [t,e] = ctxT.T @ WoT_slice
  streamed to DRAM.

Schedule: flash-style start (attention on chunk (0,0) begins right after
tchunk0's K/V/Q; later K/V groups emit inside the kt loop just before the
k-tiles that need them); b0-Q (tch 1-3) + all b1 projections drain one
item per kt into b0's attention; each chunk's finalize pipelines into the
next chunk's kt loop (stage1@kt4, out-proj@kt8/12); the final chunk reads
its PSUM accumulators directly and alternates tail copies across ACT/DVE.
Host pre-packs x/weights per-partition-contiguous (8KB/4KB DMA
descriptors) in f16; bias adds run on ACT (Identity + bias AP).
"""

import numpy as np

P = 128
D = 1024
BT = 4096
T = 2048
NB = 2
DC = 8    # D chunks of 128
TCH = 8   # 512-wide t-chunks over BT
KT = 16   # 128-wide k-tiles per batch
QC = 4    # 512-wide q-chunks per batch
NCORES = 8
DK = 64

_CACHE = {}


def _build(reps=1):
    import concourse.bass as bass
    import concourse.tile as tile
    from concourse import bacc, mybir
    from concourse.masks import make_identity

    f32 = mybir.dt.float32
    f32r = mybir.dt.float32r
    f16 = mybir.dt.float16
    Exp = mybir.ActivationFunctionType.Exp
    Copy = mybir.ActivationFunctionType.Copy
    Identity = mybir.ActivationFunctionType.Identity
    ds = bass.ds

    nc = bacc.Bacc("TRN2", target_bir_lowering=False, debug=False)

    # host-packed layouts: per-partition contiguous (big DMA descriptors)
    xt = nc.dram_tensor("xt", [P, TCH, DC, 512], f16, kind="ExternalInput").ap()
    wq = nc.dram_tensor("wq", [P, DC, P], f16, kind="ExternalInput").ap()
    wk = nc.dram_tensor("wk", [P, DC, P], f16, kind="ExternalInput").ap()
    wv = nc.dram_tensor("wv", [P, DC, P], f16, kind="ExternalInput").ap()
    wo = nc.dram_tensor("wo", [P, D], f16, kind="ExternalInput").ap()
    bqd = nc.dram_tensor("bq", [P, 1], f32, kind="ExternalInput").ap()
    bkd = nc.dram_tensor("bk", [P, 1], f32, kind="ExternalInput").ap()
    bvd = nc.dram_tensor("bv", [P, 1], f32, kind="ExternalInput").ap()
    out = nc.dram_tensor("out", [BT, D], f32, kind="ExternalOutput").ap()

    with tile.TileContext(nc) as tc:
        with (
            tc.tile_pool(name="const", bufs=1) as constp,
            tc.tile_pool(name="xtp", bufs=3) as xtp,
            tc.tile_pool(name="qkv", bufs=1) as qkvp,
            tc.tile_pool(name="vts", bufs=2) as vtsp,
            tc.tile_pool(name="esc", bufs=4) as escp,
            tc.tile_pool(name="ctx", bufs=2) as ctxp,
            tc.tile_pool(name="small", bufs=2) as smallp,
            tc.tile_pool(name="bsb", bufs=2) as bsbp,
            tc.tile_pool(name="psS", bufs=3, space="PSUM") as psS,
            tc.tile_pool(name="psC", bufs=2, space="PSUM") as psC,
        ):
            # ---- constants ----
            wq_sb = constp.tile([P, DC, P], f16, tag="wq")
            wk_sb = constp.tile([P, DC, P], f16, tag="wk")
            wv_sb = constp.tile([P, DC, P], f16, tag="wv")
            # bias DMAs are issued late (inside the rep loop) so the big
            # wk/xt transfers hit the DMA queues first
            bq_sb = constp.tile([P, 1], f32, tag="bq")
            bk_sb = constp.tile([P, 1], f32, tag="bk")
            bv_sb = constp.tile([P, 1], f32, tag="bv")
            ident_f = constp.tile([P, P], f32, tag="identf")
            make_identity(nc, ident_f)
            ident = constp.tile([P, P], f16, tag="ident")
            nc.vector.tensor_copy(ident, ident_f)
            ones_f32 = constp.tile([P, 512], f32, tag="ones_f32")
            nc.vector.memset(ones_f32, 1.0)
            ones_t = constp.tile([P, 512], f32r, tag="ones")
            nc.vector.tensor_copy(ones_t, ones_f32)
            wo_sb = constp.tile([P, D], f16, tag="wo")

            # ---- per-batch persistent tiles ----
            qt_sb = [
                qkvp.tile([P, T], f16, tag=f"qt{b}", name=f"qt{b}")
                for b in range(NB)
            ]
            kt_sb = [
                qkvp.tile([P, T], f16, tag=f"kt{b}", name=f"kt{b}")
                for b in range(NB)
            ]
            # V natural per batch per head, 65-wide blocks: [V(64) | ones]
            va_sb = [
                qkvp.tile([P, KT * 65], f16, tag=f"va{b}", name=f"va{b}")
                for b in range(NB)
            ]
            vb_sb = [
                qkvp.tile([P, KT * 65], f16, tag=f"vb{b}", name=f"vb{b}")
                for b in range(NB)
            ]
            ones_col = ones_f32[:, 0:KT].rearrange("p (k one) -> p k one", one=1)
            for b in range(NB):
                nc.vector.tensor_copy(
                    va_sb[b].rearrange("p (k c) -> p k c", c=65)[:, :, 64:65],
                    ones_col,
                )
                nc.vector.tensor_copy(
                    vb_sb[b].rearrange("p (k c) -> p k c", c=65)[:, :, 64:65],
                    ones_col,
                )

            for _rep in range(reps):

                def load_xtile(tch, split=False):
                    xtile = xtp.tile([P, DC, 512], f16, tag="xt", name="xtile")
                    if split:
                        # two DMAs so the first half-contraction of the first
                        # projection can start ~2us earlier
                        nc.sync.dma_start(xtile[:, 0:4], xt[:, tch, 0:4])
                        nc.sync.dma_start(xtile[:, 4:DC], xt[:, tch, 4:DC])
                    else:
                        nc.sync.dma_start(xtile, xt[:, tch])
                    return xtile

                def proj_w(tch, xtile, w_sb, b_sb, dst, half):
                    # half 0/1: 4 contraction chunks each; half 1 closes the
                    # accumulation group and writes dst
                    if half == 0:
                        ps = psS.tile([P, 512], f32, tag="sc", name="psw")
                        _proj_ps[(tch, id(w_sb))] = ps
                        for c in range(4):
                            nc.tensor.matmul(
                                ps, w_sb[:, c], xtile[:, c],
                                start=(c == 0), stop=False,
                            )
                    else:
                        ps = _proj_ps.pop((tch, id(w_sb)))
                        for c in range(4, DC):
                            nc.tensor.matmul(
                                ps, w_sb[:, c], xtile[:, c],
                                start=False, stop=(c == DC - 1),
                            )
                        # bias-add on ACT (Identity w/ per-partition bias):
                        # keeps the DVE free for the fin chains
                        nc.scalar.activation(dst, ps, Identity, bias=b_sb)

                def proj_v_tail(tch, vts, half):
                    # transpose VT -> V natural; 2 t-tiles per half
                    b = tch // 4
                    for tt in (0, 1) if half == 0 else (2, 3):
                        ktile = (tch % 4) * 4 + tt
                        pvt = psS.tile([P, P], f16, tag="sc", name="pvt")
                        nc.tensor.transpose(pvt, vts[:, ds(tt * P, P)], ident)
                        nc.vector.tensor_copy(
                            va_sb[b][:, ds(ktile * 65, DK)], pvt[:, 0:DK]
                        )
                        nc.vector.tensor_copy(
                            vb_sb[b][:, ds(ktile * 65, DK)], pvt[:, DK:P]
                        )

                _proj_ps = {}

                def proj_k(tch, xtile):
                    b = tch // 4
                    tloc = (tch % 4) * 512
                    dst = kt_sb[b][:, ds(tloc, 512)]
                    proj_w(tch, xtile, wk_sb, bk_sb, dst, 0)
                    proj_w(tch, xtile, wk_sb, bk_sb, dst, 1)

                def proj_v(tch, xtile):
                    vts = vtsp.tile([P, 512], f16, tag="vts", name="vts")
                    proj_w(tch, xtile, wv_sb, bv_sb, vts, 0)
                    proj_w(tch, xtile, wv_sb, bv_sb, vts, 1)
                    proj_v_tail(tch, vts, 0)
                    proj_v_tail(tch, vts, 1)

                def proj_q_thunks(tch, xtile):
                    b = tch // 4
                    tloc = (tch % 4) * 512
                    dst = qt_sb[b][:, ds(tloc, 512)]
                    return [
                        lambda: proj_w(tch, xtile, wq_sb, bq_sb, dst, 0),
                        lambda: proj_w(tch, xtile, wq_sb, bq_sb, dst, 1),
                    ]

                def proj_kv_thunks(tch, xtile):
                    b = tch // 4
                    tloc = (tch % 4) * 512
                    kdst = kt_sb[b][:, ds(tloc, 512)]
                    vts = vtsp.tile([P, 512], f16, tag="vts", name="vts")
                    return [
                        lambda: proj_w(tch, xtile, wk_sb, bk_sb, kdst, 0),
                        lambda: proj_w(tch, xtile, wk_sb, bk_sb, kdst, 1),
                        lambda: proj_w(tch, xtile, wv_sb, bv_sb, vts, 0),
                        lambda: proj_w(tch, xtile, wv_sb, bv_sb, vts, 1),
                        lambda: proj_v_tail(tch, vts, 0),
                        lambda: proj_v_tail(tch, vts, 1),
                    ]

                # pipelined finalize: stage1 (recip+bcast+normalize) and
                # stage2 (out-projection) of the previous chunk are emitted
                # inside the current chunk's kt loop to keep PE/ACT streams
                # dense.
                def fin_stage1(st, last=False):
                    # st carries ua/ub (SBUF copies) for pipelined chunks —
                    # copying frees the PSUM banks for the next chunk's
                    # accumulation. The final chunk passes the raw PSUM
                    # accumulators + the reciprocals already computed inside
                    # its attnV epilogue (no successor to stall).
                    if last:
                        b, qch, cxa, cxb, rf = st
                    else:
                        b, qch, cxa, cxb = st
                        rf = smallp.tile(
                            [P, 1024], f32, tag="recipf", name="rf"
                        )
                        # approx_fast breaks on partition-offset APs; run it
                        # over the full [0:65] range (rows 0:64 are unused
                        # junk, only the s-row 64 is consumed downstream)
                        nc.vector.reciprocal_approx_fast(
                            rf[0:65, 0:512], cxa[0:65, :]
                        )
                        nc.vector.reciprocal_approx_fast(
                            rf[0:65, 512:1024], cxb[0:65, :]
                        )
                    bc_sb = bsbp.tile([DK, 1024], f32, tag="bcs", name="bc_sb")
                    rr = smallp.tile([P, 1024], f32r, tag="recip", name="rr")
                    nc.vector.tensor_copy(rr[64:65, :], rf[64:65, :])
                    bc = psS.tile([P, 1024], f32, tag="sc", name="bc")
                    nc.tensor.matmul(
                        bc[0:DK, 0:512], ones_t[64:65, 0:DK],
                        rr[64:65, 0:512],
                        start=True, stop=True, tile_position=(64, 0),
                    )
                    nc.tensor.matmul(
                        bc[0:DK, 512:1024], ones_t[64:65, 0:DK],
                        rr[64:65, 512:1024],
                        start=True, stop=True, tile_position=(64, 0),
                    )
                    nc.vector.tensor_copy(bc_sb, bc[0:DK, :])
                    ctq = ctxp.tile([P, 512], f16, tag="ctq", name="ctq")
                    nc.vector.tensor_mul(
                        ctq[0:DK, :], cxa[0:DK, :], bc_sb[:, 0:512]
                    )
                    tmpb = bsbp.tile([DK, 512], f16, tag="tmpb", name="tmpb")
                    nc.vector.tensor_mul(
                        tmpb, cxb[0:DK, :], bc_sb[:, 512:1024]
                    )
                    nc.sync.dma_start(ctq[DK:P, :], tmpb)
                    return ctq

                def fin_stage2(st, ctq, tts, last=False):
                    b, qch = st[0], st[1]
                    del st
                    q0 = qch * 512
                    for tt in tts:
                        po = psS.tile([P, 1024], f32, tag="sc", name="po")
                        nc.tensor.matmul(
                            po[:, 0:512],
                            ctq[:, ds(tt * P, P)], wo_sb[:, 0:512],
                            start=True, stop=True,
                        )
                        nc.tensor.matmul(
                            po[:, 512:1024],
                            ctq[:, ds(tt * P, P)], wo_sb[:, 512:1024],
                            start=True, stop=True,
                        )
                        po_sb = escp.tile(
                            [P, 1024], f32, tag="posb", name="po_sb", bufs=3
                        )
                        if last and tt % 2 == 0:
                            # after the final exp the ACT engine is idle:
                            # alternate the tail's PSUM->SBUF copies across
                            # ACT and DVE to halve the serial drain
                            nc.scalar.activation(po_sb, po, Copy)
                        else:
                            nc.vector.tensor_copy(po_sb, po)
                        r0 = b * T + q0 + tt * P
                        nc.sync.dma_start(out[r0 : r0 + P, :], po_sb)

                pending = {"st": None, "ctq": None}

                work_q = []

                def drain_pending(kt):
                    # interleave previous chunk's finalize into this kt loop;
                    # stage1 early (the Pool-broadcast chain has DMA latency),
                    # out-projections late enough for ctq to be ready
                    if pending["st"] is not None:
                        if kt == 4:
                            pending["ctq"] = fin_stage1(pending["st"])
                            return
                        elif kt in (8, 10, 12, 14):
                            # one out-proj tile per slot: ~0.4us of PE work
                            # each, sized to the per-kt ACT slack so the PE
                            # stream stays dense instead of lumping
                            fin_stage2(
                                pending["st"], pending["ctq"],
                                ((kt - 8) // 2,),
                            )
                            if kt == 14:
                                pending["st"] = None
                            return
                    if kt >= 1 and work_q:
                        work_q.pop(0)()

                def attn_chunk(b, qch, interleave=None, last=False):
                    # software-pipelined by one kt: attnV(kt-1) is emitted
                    # after scores(kt)/exp(kt) so the PE never sits waiting
                    # on the exp it just triggered. `interleave` maps kt ->
                    # thunk emitted inside the loop (flash-style prologue).
                    q0 = qch * 512
                    cxa = psC.tile([65, 512], f32, tag="cx", name="cxa")
                    cxb = psC.tile([65, 512], f32, tag="cx", name="cxb")
                    escs = {}

                    def emit_av(kt, rf=None):
                        esc = escs.pop(kt)
                        nc.tensor.matmul(
                            cxa,
                            va_sb[b][:, ds(kt * 65, 65)],
                            esc[:, 0:512],
                            start=(kt == 0), stop=(kt == KT - 1),
                        )
                        if rf is not None:
                            # final chunk: head A's reciprocal runs on the
                            # DVE while head B's attnV still streams on PE
                            nc.vector.reciprocal_approx_fast(
                                rf[0:65, 0:512], cxa[0:65, :]
                            )
                        nc.tensor.matmul(
                            cxb,
                            vb_sb[b][:, ds(kt * 65, 65)],
                            esc[:, 512:1024],
                            start=(kt == 0), stop=(kt == KT - 1),
                        )
                        if rf is not None:
                            nc.vector.reciprocal_approx_fast(
                                rf[0:65, 512:1024], cxb[0:65, :]
                            )

                    for kt in range(KT):
                        sc = psS.tile([P, 1024], f32, tag="sc", name="sc")
                        nc.tensor.matmul(
                            sc[:, 0:512],
                            kt_sb[b][0:DK, ds(kt * P, P)],
                            qt_sb[b][0:DK, ds(q0, 512)],
                            start=True, stop=True,
                        )
                        nc.tensor.matmul(
                            sc[:, 512:1024],
                            kt_sb[b][DK:P, ds(kt * P, P)],
                            qt_sb[b][DK:P, ds(q0, 512)],
                            start=True, stop=True,
                            tile_position=(64, 0),
                        )
                        esc = escp.tile([P, 1024], f16, tag="esc", name="esc")
                        nc.scalar.activation(esc, sc, Exp, scale=0.125)
                        escs[kt] = esc
                        if kt > 0:
                            emit_av(kt - 1)
                        if interleave is not None and kt in interleave:
                            interleave[kt]()
                        else:
                            drain_pending(kt)
                    if last:
                        rf = smallp.tile(
                            [P, 1024], f32, tag="recipf", name="rf"
                        )
                        emit_av(KT - 1, rf=rf)
                        return (b, qch, cxa, cxb, rf)
                    emit_av(KT - 1)
                    ua = bsbp.tile([65, 512], f32, tag="ua", name="ua")
                    nc.vector.tensor_copy(ua, cxa)
                    ub = bsbp.tile([65, 512], f32, tag="ub", name="ub")
                    nc.vector.tensor_copy(ub, cxb)
                    return (b, qch, ua, ub)

                # flash-style start: K/V/Q of tchunk0 only, then begin
                # attention on chunk (0,0) immediately; K/V of tchunks 1-3
                # are emitted inside the kt loop right before the k-tiles
                # that need them (kt 4g needs tchunk g).
                xtiles_b0 = {}
                if _rep == 0:
                    nc.sync.dma_start(wk_sb, wk)
                xtiles_b0[0] = load_xtile(0, split=True)
                if _rep == 0:
                    nc.sync.dma_start(wv_sb, wv)
                    nc.sync.dma_start(wq_sb, wq)
                    nc.sync.dma_start(bk_sb, bkd)
                    nc.sync.dma_start(bv_sb, bvd)
                    nc.sync.dma_start(bq_sb, bqd)
                proj_k(0, xtiles_b0[0])
                xtiles_b0[1] = load_xtile(1)
                proj_v(0, xtiles_b0[0])
                for th in proj_q_thunks(0, xtiles_b0[0]):
                    th()
                if _rep == 0:
                    nc.sync.dma_start(wo_sb, wo)

                # K/V groups split in half so the ACT engine's ~2-exp
                # lookahead covers each PE burst: K(g) lands the kt before
                # its first score needs it, V(g) one kt later (attnV runs
                # one kt behind scores)
                def flash_k(g):
                    def th():
                        if g + 1 < 4:
                            xtiles_b0[g + 1] = load_xtile(g + 1)
                        proj_k(g, xtiles_b0[g])
                    return th

                def flash_v(g):
                    def th():
                        proj_v(g, xtiles_b0[g])
                    return th

                flash = {
                    3: flash_k(1), 4: flash_v(1),
                    7: flash_k(2), 8: flash_v(2),
                    11: flash_k(3), 12: flash_v(3),
                }

                # defer b0 Q (tch 1-3) + all of b1's projections into b0's
                # attention. Q thunks resolve xtiles_b0[tch] lazily (tch 2/3
                # xtiles only load inside the flash interleave).
                def q_b0(tch, half):
                    def th():
                        proj_q_thunks(tch, xtiles_b0[tch])[half]()
                    return th

                xt_b1 = {}

                def mk_load(tch):
                    def th():
                        xt_b1[tch] = load_xtile(tch)
                    return th

                def mk_body(tch, n=8):
                    out = []
                    for i in range(min(n, 6)):
                        def th(tch=tch, i=i):
                            xti = xt_b1[tch]
                            if "thunks" not in xt_b1.setdefault(
                                f"t{tch}", {}
                            ):
                                xt_b1[f"t{tch}"]["thunks"] = (
                                    proj_kv_thunks(tch, xti)
                                    + proj_q_thunks(tch, xti)
                                )
                            xt_b1[f"t{tch}"]["thunks"][i]()
                        out.append(th)
                    for i in range(max(0, n - 6)):
                        def th2(tch=tch, i=i):
                            xt_b1[f"t{tch}"]["thunks"][6 + i]()
                        out.append(th2)
                    return out

                def mk_q(tch, half):
                    def th():
                        xt_b1[f"t{tch}"]["thunks"][6 + half]()
                    return th

                work_q.append(q_b0(1, 0))
                work_q.append(q_b0(1, 1))
                work_q.append(q_b0(2, 0))
                work_q.append(q_b0(2, 1))
                work_q.append(mk_load(4))
                work_q.append(mk_load(5))
                work_q.append(q_b0(3, 0))
                work_q.append(q_b0(3, 1))
                work_q.extend(mk_body(4))
                work_q.append(mk_load(6))
                work_q.extend(mk_body(5))
                work_q.append(mk_load(7))
                work_q.extend(mk_body(6))
                work_q.extend(mk_body(7))
                pending["st"] = attn_chunk(0, 0, interleave=flash)
                for qch in range(1, 4):
                    pending["st"] = attn_chunk(0, qch)
                while work_q:
                    work_q.pop(0)()
                for qch in range(3):
                    pending["st"] = attn_chunk(1, qch)
                pending["st"] = attn_chunk(1, 3, last=True)
                ctq = fin_stage1(pending["st"], last=True)
                fin_stage2(pending["st"], ctq, (0, 1, 2, 3), last=True)

    nc.compile()
    return nc


def _get_nc(reps=1):
    key = f"nc{reps}"
    if key not in _CACHE:
        _CACHE[key] = _build(reps)
    return _CACHE[key]


def kernel(x, Wq, bq, Wk, bk, Wv, bv, Wo, bo):
    from concourse.bass_utils import run_bass_kernel_spmd

    x = np.asarray(x, dtype=np.float32)
    Wq = np.asarray(Wq, dtype=np.float32)
    Wk = np.asarray(Wk, dtype=np.float32)
    Wv = np.asarray(Wv, dtype=np.float32)
    Wo = np.asarray(Wo, dtype=np.float32)
    bq = np.asarray(bq, dtype=np.float32)
    bk = np.asarray(bk, dtype=np.float32)
    bv = np.asarray(bv, dtype=np.float32)
    bo = np.asarray(bo, dtype=np.float32)

    B, Tl, Dl = x.shape
    x_flat = x.reshape(B * Tl, Dl)
    # [p, tch, c, tt] with value x_flat[tch*512 + tt, c*128 + p]
    xt = np.ascontiguousarray(
        x_flat.reshape(TCH, 512, DC, P).transpose(3, 0, 2, 1)
    ).astype(np.float16)

    def pack_w(Wslice_T):
        # [p, c, e] with value Wslice_T[c*128 + p, e]
        return np.ascontiguousarray(
            Wslice_T.reshape(DC, P, P).transpose(1, 0, 2)
        ).astype(np.float16)

    in_maps = []
    for c in range(NCORES):
        sl = slice(c * P, (c + 1) * P)
        in_maps.append(
            {
                "xt": xt,
                "wq": pack_w(np.ascontiguousarray(Wq[sl, :].T)),
                "wk": pack_w(np.ascontiguousarray(Wk[sl, :].T)),
                "wv": pack_w(np.ascontiguousarray(Wv[sl, :].T)),
                "wo": np.ascontiguousarray(Wo[:, sl].T).astype(np.float16),
                "bq": np.ascontiguousarray(bq[sl].reshape(P, 1)),
                "bk": np.ascontiguousarray(bk[sl].reshape(P, 1)),
                "bv": np.ascontiguousarray(bv[sl].reshape(P, 1)),
            }
        )

    nc = _get_nc()
    _CACHE["in_maps"] = in_maps
    res = run_bass_kernel_spmd(nc, in_maps, core_ids=list(range(NCORES)))
    acc = res.results[0]["out"].astype(np.float32)
    for c in range(1, NCORES):
        acc = acc + res.results[c]["out"]
    acc = acc + bo[None, :]
    return acc.reshape(B, Tl, Dl).astype(np.float32)
